# revision 39
# baseline (speedup 1.0000x reference)
"""Trainium2 Bass kernel: BasicMultiheadAttention (B=2, S=2048, D=1024, H=16).

Sharding: tensor-parallel over heads (core c owns heads 2c, 2c+1 for both
batches); per-query-chunk AllGather of normalized ctx^T across the 8 cores;
column-sharded output projection (+bias) per core.

Softmax exp is split across two engines per key tile: head 0 on ACT (true
exp, PSUM->SBUF f16), head 1 on DVE via a Schraudolph bit-trick exp
(int16(A*x+B) bitcast as f16), with the PV matmuls lagged two units so the
in-order PE never waits on exp.  The softmax denominator rides in the PV
matmul as a ones column (row 64 of ctx psum); normalization is one
reciprocal_approx_fast over a staged [128,512] tile (rows 0/64 hold the two
heads' denominator rows), one E-selector matmul that broadcasts both
reciprocals across partitions, one PSUM->SBUF copy, and two DVE multiplies.

Batch 1's QKV chunks are interleaved into batch 0's attention as PE filler
so ACT/DVE exp throughput, not the PE, sets the attention pace.  The final
query chunk's AllGather is split in two halves to shorten the tail.
"""

import numpy as np

B, S, D, H = 2, 2048, 1024, 16
DH = D // H  # 64
NCORES = 8
HPC = H // NCORES  # heads per core = 2
SQ = B * S  # 4096 tokens
NKT = D // 128  # 8 contraction k-tiles over D
KT_S = S // 128  # 16 key tiles per batch
QC_S = S // 512  # 4 query chunks of 512 per batch
NHB = 4  # half-batches (a2a granularity): 1024 tokens each
TOKCH = 128  # tokens per core per half-batch after AllToAll

# Schraudolph exp-as-f16-bits: bits = round(1024*(x*0.125*log2e + 15 - sigma))
_SCH_A = 0.125 * np.log2(np.e) * 1024.0
_SCH_B = (15.0 - 0.0430) * 1024.0
# h0 exp runs on ACT; h1 exp runs on DVE (Schraudolph) for these key tiles,
# on ACT otherwise.  Concurrent engines halve the exp critical path.
_DVE_H1_KT = set(range(16))

_CACHE = {}


def _ensure_axon_hooks():
    """This image's antenv lacks axon_hooks; bass_utils imports it when
    trace=True under axon. Register an equivalent stub backed by the boot
    helper so NTFF profiling works (or degrades gracefully)."""
    import sys
    import types
    try:
        import antenv.axon_hooks  # noqa: F401
        return
    except ImportError:
        pass
    try:
        import antenv
        hook = [None]
        try:
            from trn_agent_boot.trn_boot import _ntff_profile_via_ctypes
            hook[0] = _ntff_profile_via_ctypes("/opt/axon/libaxon_pjrt.so")
        except Exception:
            hook[0] = None
        mod = types.ModuleType("antenv.axon_hooks")
        mod.get_axon_ntff_profile_hook = lambda: hook[0]
        mod.set_axon_ntff_profile_hook = lambda h: hook.__setitem__(0, h)
        sys.modules["antenv.axon_hooks"] = mod
        antenv.axon_hooks = mod
    except Exception:
        pass


_ensure_axon_hooks()


def _build_kernel():
    import concourse.bass as bass  # noqa: F401
    import concourse.mybir as mybir
    import concourse.tile as tile
    from concourse import bacc
    from concourse.masks import make_identity

    f16 = mybir.dt.float16
    f32 = mybir.dt.float32
    f32r = mybir.dt.float32r
    i16 = mybir.dt.int16
    AF = mybir.ActivationFunctionType
    ALU = mybir.AluOpType

    nc = bacc.Bacc(None, num_devices=NCORES)

    # ---- I/O ----
    xT = nc.dram_tensor("xT", [D, SQ], f16, kind="ExternalInput")
    # wq|wk|wv (per-core heads, kt-major) then per-core Wo slice (kt-major)
    wpack = nc.dram_tensor("wpack", [128, 4 * D], f16, kind="ExternalInput")
    bpack = nc.dram_tensor("bpack", [128, 4], f32, kind="ExternalInput")
    # column-sharded output: yT[p, t] = out[token t, dim 128*core+p]
    yT = nc.dram_tensor("yT", [128, SQ], f32, kind="ExternalOutput")

    with tile.TileContext(nc) as tc:
        with (
            tc.tile_pool(name="const", bufs=1) as const,
            tc.tile_pool(name="psSc", bufs=4, space="PSUM") as psSc,
            tc.tile_pool(name="psCtx", bufs=2, space="PSUM") as psCtx,
            tc.tile_pool(name="psO", bufs=2, space="PSUM") as psO,
            tc.tile_pool(name="pP", bufs=4) as pP,
            tc.tile_pool(name="pDen", bufs=2) as pDen,
            tc.tile_pool(name="pCg", bufs=2) as pCg,
            tc.tile_pool(name="pOut", bufs=2) as pOut,
            tc.tile_pool(name="dram", bufs=1, space="DRAM") as dram,
        ):
            # ---- constants / small DMAs first (qkv weights first) ----
            wpack_sb = const.tile([128, 4 * D], f16)
            bpack_sb = const.tile([128, 4], f32)
            nc.sync.dma_start(bpack_sb[:], bpack[:, :])
            nc.sync.dma_start(wpack_sb[:, 0:3 * D], wpack[:, 0:3 * D])
            wq_sb = wpack_sb[:, 0 * D:1 * D]
            wk_sb = wpack_sb[:, 1 * D:2 * D]
            wv_sb = wpack_sb[:, 2 * D:3 * D]
            wo_sb = wpack_sb[:, 3 * D:4 * D]  # [128, NKT*128] kt-major
            bq_sb = bpack_sb[:, 0:1]
            bk_sb = bpack_sb[:, 1:2]
            bvt_sb = bpack_sb[:, 2:3]
            bo_sb = bpack_sb[:, 3:4]

            ident = const.tile([128, 128], f16)
            make_identity(nc, ident)

            # E_comb: out partition m gets rec row 0 (m<64) or row 64
            # (m>=64) when used as matmul weights on rec16c.
            ecomb = const.tile([128, 128], f16)
            nc.vector.memset(ecomb[:], 0.0)
            nc.vector.memset(ecomb[0:1, 0:64], 1.0)
            nc.vector.memset(ecomb[64:65, 64:128], 1.0)

            # reciprocal staging: rows 0/64 get the two heads' denominators,
            # other rows stay 1.0 so recip_approx_fast stays finite there
            # (the op misbehaves on <128-partition APs).
            rec_st = const.tile([128, 512], f32, name="rec_st")
            nc.vector.memset(rec_st[:], 1.0)
            rec_full = const.tile([128, 512], f32, name="rec_full")
            rec16c = const.tile([128, 512], f16, name="rec16c")
            nc.vector.memset(rec16c[:], 0.0)

            # ---- xT load: one 3D-AP DMA per 512-token segment, b0 first;
            # Wo weights slot in after b0's segments.
            xt_sb = const.tile([128, NKT * SQ], f16)
            xt_v = xt_sb.rearrange("p (k t) -> p k t", k=NKT)
            xT_v = xT.rearrange("(k p) t -> p k t", p=128)
            for seg in range(4):
                t0 = seg * 512
                nc.sync.dma_start(
                    xt_v[:, :, t0:t0 + 512],
                    xT_v[:, :, t0:t0 + 512],
                )
            nc.sync.dma_start(wpack_sb[:, 3 * D:4 * D], wpack[:, 3 * D:4 * D])
            for seg in range(4, 8):
                t0 = seg * 512
                nc.sync.dma_start(
                    xt_v[:, :, t0:t0 + 512],
                    xT_v[:, :, t0:t0 + 512],
                )

            qT_sb = const.tile([128, SQ], f16)
            kT_sb = const.tile([128, SQ], f16)
            vT_sb = const.tile([128, SQ], f16)
            # V with ones column: per (b, head, key-tile) a [128, 65] region
            NREG = B * HPC * KT_S  # 64 regions
            vaug_sb = const.tile([128, NREG * 65], f16)
            ones_cols = vaug_sb.rearrange("p (r c) -> p r c", c=65)[:, :, 64:65]
            nc.vector.memset(ones_cols, 1.0)

            ctxT_sb = const.tile([128, SQ], f16)

            # warmup collective: absorb ncfw first-trigger init early
            wu_loc = dram.tile([128, 2], f16, name="wu_loc")
            nc.sync.dma_start(wu_loc[:], ctxT_sb[:, 0:2])
            wu_g = dram.tile([NCORES * 128, 2], f16, addr_space="Shared",
                             name="wu_g")
            nc.gpsimd.collective_compute(
                "AllGather", mybir.AluOpType.bypass,
                replica_groups=[list(range(NCORES))],
                ins=[wu_loc.opt()], outs=[wu_g.opt()],
            )

            # ---- QKV helpers ----
            def emit_qkv_chunk(b, proj, ncx):
                w_sb, b_sb, dst = (
                    (wq_sb, bq_sb, qT_sb), (wk_sb, bk_sb, kT_sb),
                    (wv_sb, bvt_sb, vT_sb),
                )[proj]
                tok0 = b * S + ncx * 512
                ps = psO.tile([128, 512], f32, tag="a",
                              name=f"qkv_{b}_{proj}_{ncx}")
                for kt in range(NKT):
                    nc.tensor.matmul(
                        ps[:],
                        lhsT=w_sb[:, kt * 128:(kt + 1) * 128],
                        rhs=xt_sb[:, kt * SQ + tok0: kt * SQ + tok0 + 512],
                        start=(kt == 0),
                        stop=(kt == NKT - 1),
                    )
                nc.scalar.activation(dst[:, tok0:tok0 + 512], ps[:],
                                     AF.Identity, bias=b_sb)

            def emit_vaug(b):
                # transpose V^T tiles into [tokens, dims] regions
                for tt in range(KT_S):
                    tok0 = b * S + tt * 128
                    vtr = psO.tile([128, 128], f16, tag="a",
                                   name=f"vtr_{b}_{tt}")
                    nc.tensor.transpose(vtr[:], vT_sb[:, tok0:tok0 + 128],
                                        ident[:])
                    for h in range(HPC):
                        r = (b * HPC + h) * KT_S + tt
                        nc.vector.tensor_copy(
                            vaug_sb[:, r * 65: r * 65 + 64],
                            vtr[:, h * 64:(h + 1) * 64],
                        )

            # ---- attention inner loop ----
            def emit_attention_qc(b, qc, half=None):
                q0 = b * S + qc * 512
                w = 512 if half is None else 256
                if half:
                    q0 += 256
                htag = "" if half is None else f"_{half}"
                ctx_ps = [
                    psCtx.tile([65, 512], f32, tag="ctx",
                               name=f"ctx_{b}_{qc}_{h}{htag}")
                    for h in range(HPC)
                ]
                # Per (kt, head): scores MM -> exp on ACT (h0) / DVE (h1),
                # PV lagged 2 units so the in-order PE never waits on exp.
                def emit_pv(kt, h, p_sb):
                    r = (b * HPC + h) * KT_S + kt
                    nc.tensor.matmul(
                        ctx_ps[h][0:65, 0:w],
                        lhsT=vaug_sb[:, r * 65:(r + 1) * 65],
                        rhs=p_sb[:, h * w:(h + 1) * w],
                        start=(kt == 0),
                        stop=(kt == KT_S - 1),
                    )

                pvq = []
                for kt in range(KT_S):
                    k0 = b * S + kt * 128
                    p_sb = pP.tile([128, 1024], f16, tag="p",
                                   name=f"p_{b}_{qc}_{kt}{htag}")
                    for h in range(HPC):
                        sc = psSc.tile([128, 512], f32, tag="sc",
                                       name=f"sc_{b}_{qc}_{kt}_{h}{htag}")
                        nc.tensor.matmul(
                            sc[0:128, 0:w],
                            lhsT=kT_sb[h * 64:(h + 1) * 64, k0:k0 + 128],
                            rhs=qT_sb[h * 64:(h + 1) * 64, q0:q0 + w],
                            start=True, stop=True,
                            tile_position=(h * 64, 0),
                        )
                        if h == 1 and kt in _DVE_H1_KT:
                            nc.vector.tensor_scalar(
                                p_sb.bitcast(i16)[:, h * w:(h + 1) * w],
                                sc[0:128, 0:w], _SCH_A, _SCH_B,
                                ALU.mult, ALU.add,
                            )
                        else:
                            nc.scalar.activation(
                                p_sb[:, h * w:(h + 1) * w], sc[0:128, 0:w],
                                AF.Exp, scale=0.125)
                        pvq.append((kt, h, p_sb))
                        if len(pvq) > 2:
                            emit_pv(*pvq.pop(0))
                for it in pvq:
                    emit_pv(*it)
                # normalize: denom row 64 -> recip -> broadcast -> multiply
                rps = psO.tile([128, 512], f32, tag="a",
                               name=f"rps_{b}_{qc}{htag}")
                for h in range(HPC):
                    nc.scalar.copy(rec_st[h * 64:h * 64 + 1, 0:w],
                                   ctx_ps[h][64:65, 0:w])
                nc.vector.reciprocal_approx_fast(rec_full[0:128, 0:w],
                                                 rec_st[0:128, 0:w])
                for h in range(HPC):
                    nc.vector.tensor_copy(rec16c[h * 64:h * 64 + 1, 0:w],
                                          rec_full[h * 64:h * 64 + 1, 0:w])
                nc.tensor.matmul(rps[0:128, 0:w], lhsT=ecomb[:],
                                 rhs=rec16c[0:128, 0:w],
                                 start=True, stop=True)
                rps_sb = pDen.tile([128, 512], f16, tag="d",
                                   name=f"rd_{b}_{qc}{htag}")
                nc.scalar.copy(rps_sb[0:128, 0:w], rps[0:128, 0:w])
                for h in range(HPC):
                    nc.vector.tensor_tensor(
                        ctxT_sb[h * 64:(h + 1) * 64, q0:q0 + w],
                        ctx_ps[h][0:64, 0:w],
                        rps_sb[h * 64:(h + 1) * 64, 0:w], ALU.mult,
                    )

            # ---- per-qc AllGather + column-sharded output projection ----
            # The last qc is gathered/projected in two 256-token halves to
            # shorten the end-of-kernel serial tail.
            gath = {}

            def emit_gather(b, qc, half=None):
                q0 = b * S + qc * 512
                w = 512 if half is None else 256
                if half:
                    q0 += 256
                tag = f"{b}_{qc}" + ("" if half is None else f"_{half}")
                g_in = dram.tile([128, w], f16, name=f"gin_{tag}")
                nc.sync.dma_start(g_in[:], ctxT_sb[:, q0:q0 + w])
                g_out = dram.tile([NCORES * 128, w], f16,
                                  addr_space="Shared", name=f"gout_{tag}")
                nc.gpsimd.collective_compute(
                    "AllGather", mybir.AluOpType.bypass,
                    replica_groups=[list(range(NCORES))],
                    ins=[g_in.opt()], outs=[g_out.opt()],
                )
                gath[(b, qc, half)] = g_out

            def emit_outproj(key):
                b, qc, half = key
                q0 = b * S + qc * 512
                w = 512 if half is None else 256
                if half:
                    q0 += 256
                g_out = gath[key]
                tag = f"{b}_{qc}" + ("" if half is None else f"_{half}")
                cg = pCg.tile([128, NKT, 512], f16, tag="cg",
                              name=f"cg_{tag}")
                nc.sync.dma_start(
                    cg[:, :, 0:w],
                    g_out.rearrange("(k p) q -> p k q", p=128),
                )
                po = psO.tile([128, 512], f32, tag="a", name=f"po_{tag}")
                for k in range(NKT):
                    nc.tensor.matmul(
                        po[:, 0:w],
                        lhsT=wo_sb[:, k * 128:(k + 1) * 128],
                        rhs=cg[:, k, 0:w],
                        start=(k == 0),
                        stop=(k == NKT - 1),
                    )
                out_sb = pOut.tile([128, 512], f32, tag="os",
                                   name=f"os_{tag}")
                nc.vector.tensor_scalar(out_sb[:, 0:w], po[:, 0:w], bo_sb,
                                        None, ALU.add)
                nc.sync.dma_start(yT[:, q0:q0 + w], out_sb[:, 0:w])

            # ---- schedule ----
            # QKV b0 fully, then b0 attention with b1's QKV interleaved.
            for proj in range(3):
                for ncx in range(QC_S):
                    emit_qkv_chunk(0, proj, ncx)
            emit_vaug(0)

            fillers = ([(1, 2, n) for n in range(QC_S)]
                       + [(1, 1, n) for n in range(QC_S)]
                       + [(1, 0, n) for n in range(QC_S)])
            per_qc = [3, 3, 3, 3]
            pending_outproj = []
            for qc in range(QC_S):
                emit_attention_qc(0, qc)
                emit_gather(0, qc)
                pending_outproj.append((0, qc, None))
                for _ in range(per_qc[qc]):
                    if fillers:
                        b_, p_, n_ = fillers.pop(0)
                        emit_qkv_chunk(b_, p_, n_)
                if qc == 2:
                    emit_vaug(1)
                if len(pending_outproj) > 1:
                    emit_outproj(pending_outproj.pop(0))
            for qc in range(QC_S):
                emit_attention_qc(1, qc)
                if qc < QC_S - 1:
                    emit_gather(1, qc)
                    pending_outproj.append((1, qc, None))
                else:
                    emit_gather(1, qc, 0)
                    emit_gather(1, qc, 1)
                    pending_outproj += [(1, qc, 0), (1, qc, 1)]
                if len(pending_outproj) > 1:
                    emit_outproj(pending_outproj.pop(0))
            for key in pending_outproj:
                emit_outproj(key)

    nc.finalize()
    return nc


def kernel(x, Wq, Wk, Wv, bq, bk, bv, Wo, bo):
    from concourse.bass_utils import run_bass_kernel_spmd

    if "nc" not in _CACHE:
        _CACHE["nc"] = _build_kernel()
    nc = _CACHE["nc"]

    # host-side prep
    xTh = np.ascontiguousarray(
        x.astype(np.float32).transpose(2, 0, 1).reshape(D, SQ)
    ).astype(np.float16)

    def pack_w(Wslice):
        # [D, M] -> [128, NKT*M] kt-major: out[p, kt*M+m] = Wslice[kt*128+p, m]
        M = Wslice.shape[1]
        return np.ascontiguousarray(
            Wslice.reshape(NKT, 128, M).transpose(1, 0, 2).reshape(128, NKT * M)
        ).astype(np.float16)

    in_maps = []
    for c in range(NCORES):
        hA, hB = HPC * c, HPC * c + 1
        wq_c = pack_w(np.concatenate([Wq[hA], Wq[hB]], axis=1))
        wk_c = pack_w(np.concatenate([Wk[hA], Wk[hB]], axis=1))
        wv_c = pack_w(np.concatenate([Wv[hA], Wv[hB]], axis=1))
        wo_c = pack_w(Wo[:, 128 * c:128 * (c + 1)])
        wpack_c = np.ascontiguousarray(
            np.concatenate([wq_c, wk_c, wv_c, wo_c], axis=1))
        bq_c = np.concatenate([bq[hA], bq[hB]]).reshape(128, 1)
        bk_c = np.concatenate([bk[hA], bk[hB]]).reshape(128, 1)
        bv_c = np.concatenate([bv[hA], bv[hB]]).reshape(128, 1)
        bo_c = bo[128 * c:128 * (c + 1)].reshape(128, 1)
        bpack_c = np.ascontiguousarray(
            np.concatenate([bq_c, bk_c, bv_c, bo_c],
                           axis=1)).astype(np.float32)
        in_maps.append({"xT": xTh, "wpack": wpack_c, "bpack": bpack_c})

    res = run_bass_kernel_spmd(nc, in_maps, core_ids=list(range(NCORES)))
    _CACHE["last_result"] = res
    # assemble: core c's yT [128, SQ] are output columns 128c..128c+127
    out = np.empty((B, S, D), dtype=np.float32)
    for c in range(NCORES):
        yt = res.results[c]["yT"]  # [128, SQ]
        out[:, :, 128 * c:128 * (c + 1)] = (
            yt.reshape(128, B, S).transpose(1, 2, 0)
        )
    return out


# revision 40
# speedup vs baseline: 1.0076x; 1.0076x over previous
"""Trainium2 Bass kernel: BasicMultiheadAttention (B=2, S=2048, D=1024, H=16).

Sharding: tensor-parallel over heads (core c owns heads 2c, 2c+1 for both
batches); per-query-chunk AllGather of normalized ctx^T across the 8 cores;
column-sharded output projection (+bias) per core.

Softmax exp is split across two engines per key tile: head 0 on ACT (true
exp, PSUM->SBUF f16), head 1 on DVE via a Schraudolph bit-trick exp
(int16(A*x+B) bitcast as f16), with the PV matmuls lagged two units so the
in-order PE never waits on exp.  The softmax denominator rides in the PV
matmul as a ones column (row 64 of ctx psum); normalization is one
reciprocal_approx_fast over a staged [128,512] tile (rows 0/64 hold the two
heads' denominator rows), one E-selector matmul that broadcasts both
reciprocals across partitions, one PSUM->SBUF copy, and two DVE multiplies.

Batch 1's QKV chunks are interleaved into batch 0's attention as PE filler
so ACT/DVE exp throughput, not the PE, sets the attention pace.  The final
query chunk's AllGather is split in two halves to shorten the tail.
"""

import numpy as np

B, S, D, H = 2, 2048, 1024, 16
DH = D // H  # 64
NCORES = 8
HPC = H // NCORES  # heads per core = 2
SQ = B * S  # 4096 tokens
NKT = D // 128  # 8 contraction k-tiles over D
KT_S = S // 128  # 16 key tiles per batch
QC_S = S // 512  # 4 query chunks of 512 per batch

# Schraudolph exp-as-f16-bits: bits = round(1024*(x*0.125*log2e + 15 - sigma))
_SCH_A = 0.125 * np.log2(np.e) * 1024.0
_SCH_B = (15.0 - 0.0430) * 1024.0
# h0 exp runs on ACT; h1 exp runs on DVE (Schraudolph) for these key tiles,
# on ACT otherwise.  Concurrent engines halve the exp critical path.
_DVE_H1_KT = set(range(16))

_CACHE = {}


def _ensure_axon_hooks():
    """This image's antenv lacks axon_hooks; bass_utils imports it when
    trace=True under axon. Register an equivalent stub backed by the boot
    helper so NTFF profiling works (or degrades gracefully)."""
    import sys
    import types
    try:
        import antenv.axon_hooks  # noqa: F401
        return
    except ImportError:
        pass
    try:
        import antenv
        hook = [None]
        try:
            from trn_agent_boot.trn_boot import _ntff_profile_via_ctypes
            hook[0] = _ntff_profile_via_ctypes("/opt/axon/libaxon_pjrt.so")
        except Exception:
            hook[0] = None
        mod = types.ModuleType("antenv.axon_hooks")
        mod.get_axon_ntff_profile_hook = lambda: hook[0]
        mod.set_axon_ntff_profile_hook = lambda h: hook.__setitem__(0, h)
        sys.modules["antenv.axon_hooks"] = mod
        antenv.axon_hooks = mod
    except Exception:
        pass


_ensure_axon_hooks()


def _build_kernel():
    import concourse.bass as bass  # noqa: F401
    import concourse.mybir as mybir
    import concourse.tile as tile
    from concourse import bacc
    from concourse.masks import make_identity

    f16 = mybir.dt.float16
    f32 = mybir.dt.float32
    f32r = mybir.dt.float32r
    i16 = mybir.dt.int16
    AF = mybir.ActivationFunctionType
    ALU = mybir.AluOpType

    nc = bacc.Bacc(None, num_devices=NCORES)

    # ---- I/O ----
    xT = nc.dram_tensor("xT", [D, SQ], f16, kind="ExternalInput")
    # wq|wk|wv (per-core heads, kt-major) then per-core Wo slice (kt-major)
    wpack = nc.dram_tensor("wpack", [128, 4 * D], f16, kind="ExternalInput")
    bpack = nc.dram_tensor("bpack", [128, 4], f32, kind="ExternalInput")
    # column-sharded output: yT[p, t] = out[token t, dim 128*core+p]
    yT = nc.dram_tensor("yT", [128, SQ], f32, kind="ExternalOutput")

    with tile.TileContext(nc) as tc:
        with (
            tc.tile_pool(name="const", bufs=1) as const,
            tc.tile_pool(name="psSc", bufs=4, space="PSUM") as psSc,
            tc.tile_pool(name="psCtx", bufs=2, space="PSUM") as psCtx,
            tc.tile_pool(name="psO", bufs=2, space="PSUM") as psO,
            tc.tile_pool(name="pP", bufs=4) as pP,
            tc.tile_pool(name="pDen", bufs=2) as pDen,
            tc.tile_pool(name="pCg", bufs=2) as pCg,
            tc.tile_pool(name="pOut", bufs=2) as pOut,
            tc.tile_pool(name="dram", bufs=1, space="DRAM") as dram,
        ):
            # ---- constants / small DMAs first (qkv weights first) ----
            wpack_sb = const.tile([128, 4 * D], f16)
            bpack_sb = const.tile([128, 4], f32)
            nc.sync.dma_start(bpack_sb[:], bpack[:, :])
            nc.sync.dma_start(wpack_sb[:, 0:3 * D], wpack[:, 0:3 * D])
            wq_sb = wpack_sb[:, 0 * D:1 * D]
            wk_sb = wpack_sb[:, 1 * D:2 * D]
            wv_sb = wpack_sb[:, 2 * D:3 * D]
            wo_sb = wpack_sb[:, 3 * D:4 * D]  # [128, NKT*128] kt-major
            bq_sb = bpack_sb[:, 0:1]
            bk_sb = bpack_sb[:, 1:2]
            bvt_sb = bpack_sb[:, 2:3]
            bo_sb = bpack_sb[:, 3:4]

            ident = const.tile([128, 128], f16)
            make_identity(nc, ident)

            # E_comb: out partition m gets rec row 0 (m<64) or row 64
            # (m>=64) when used as matmul weights on rec16c.
            ecomb = const.tile([128, 128], f16)
            nc.vector.memset(ecomb[:], 0.0)
            nc.vector.memset(ecomb[0:1, 0:64], 1.0)
            nc.vector.memset(ecomb[64:65, 64:128], 1.0)

            # reciprocal staging: rows 0/64 get the two heads' denominators,
            # other rows stay 1.0 so recip_approx_fast stays finite there
            # (the op misbehaves on <128-partition APs).
            rec_st = const.tile([128, 512], f32, name="rec_st")
            nc.vector.memset(rec_st[:], 1.0)
            rec_full = const.tile([128, 512], f32, name="rec_full")
            rec16c = const.tile([128, 512], f16, name="rec16c")
            nc.vector.memset(rec16c[:], 0.0)

            # ---- xT load: one 3D-AP DMA per 512-token segment, b0 first;
            # Wo weights slot in after b0's segments.
            xt_sb = const.tile([128, NKT * SQ], f16)
            xt_v = xt_sb.rearrange("p (k t) -> p k t", k=NKT)
            xT_v = xT.rearrange("(k p) t -> p k t", p=128)
            for seg in range(4):
                t0 = seg * 512
                nc.sync.dma_start(
                    xt_v[:, :, t0:t0 + 512],
                    xT_v[:, :, t0:t0 + 512],
                )
            nc.sync.dma_start(wpack_sb[:, 3 * D:4 * D], wpack[:, 3 * D:4 * D])
            for seg in range(4, 8):
                t0 = seg * 512
                nc.sync.dma_start(
                    xt_v[:, :, t0:t0 + 512],
                    xT_v[:, :, t0:t0 + 512],
                )

            qT_sb = const.tile([128, SQ], f16)
            kT_sb = const.tile([128, SQ], f16)
            vT_sb = const.tile([128, SQ], f16)
            # V with ones column: per (b, head, key-tile) a [128, 65] region
            NREG = B * HPC * KT_S  # 64 regions
            vaug_sb = const.tile([128, NREG * 65], f16)
            ones_cols = vaug_sb.rearrange("p (r c) -> p r c", c=65)[:, :, 64:65]
            nc.vector.memset(ones_cols, 1.0)

            ctxT_sb = const.tile([128, SQ], f16)

            # warmup collective: absorb ncfw first-trigger init early
            wu_loc = dram.tile([128, 2], f16, name="wu_loc")
            nc.sync.dma_start(wu_loc[:], ctxT_sb[:, 0:2])
            wu_g = dram.tile([NCORES * 128, 2], f16, addr_space="Shared",
                             name="wu_g")
            nc.gpsimd.collective_compute(
                "AllGather", mybir.AluOpType.bypass,
                replica_groups=[list(range(NCORES))],
                ins=[wu_loc.opt()], outs=[wu_g.opt()],
            )

            # ---- QKV helpers ----
            def emit_qkv_chunk(b, proj, ncx):
                w_sb, b_sb, dst = (
                    (wq_sb, bq_sb, qT_sb), (wk_sb, bk_sb, kT_sb),
                    (wv_sb, bvt_sb, vT_sb),
                )[proj]
                tok0 = b * S + ncx * 512
                ps = psO.tile([128, 512], f32, tag="a",
                              name=f"qkv_{b}_{proj}_{ncx}")
                for kt in range(NKT):
                    nc.tensor.matmul(
                        ps[:],
                        lhsT=w_sb[:, kt * 128:(kt + 1) * 128],
                        rhs=xt_sb[:, kt * SQ + tok0: kt * SQ + tok0 + 512],
                        start=(kt == 0),
                        stop=(kt == NKT - 1),
                    )
                nc.scalar.activation(dst[:, tok0:tok0 + 512], ps[:],
                                     AF.Identity, bias=b_sb)

            def emit_vaug(b):
                # transpose V^T tiles into [tokens, dims] regions
                for tt in range(KT_S):
                    tok0 = b * S + tt * 128
                    vtr = psO.tile([128, 128], f16, tag="a",
                                   name=f"vtr_{b}_{tt}")
                    nc.tensor.transpose(vtr[:], vT_sb[:, tok0:tok0 + 128],
                                        ident[:])
                    for h in range(HPC):
                        r = (b * HPC + h) * KT_S + tt
                        nc.vector.tensor_copy(
                            vaug_sb[:, r * 65: r * 65 + 64],
                            vtr[:, h * 64:(h + 1) * 64],
                        )

            # ---- attention inner loop ----
            def emit_attention_qc(b, qc, half=None):
                q0 = b * S + qc * 512
                w = 512 if half is None else 256
                if half:
                    q0 += 256
                htag = "" if half is None else f"_{half}"
                ctx_ps = [
                    psCtx.tile([65, 512], f32, tag="ctx",
                               name=f"ctx_{b}_{qc}_{h}{htag}")
                    for h in range(HPC)
                ]
                # Per (kt, head): scores MM -> exp on ACT (h0) / DVE (h1),
                # PV lagged 2 units so the in-order PE never waits on exp.
                def emit_pv(kt, h, p_sb):
                    r = (b * HPC + h) * KT_S + kt
                    nc.tensor.matmul(
                        ctx_ps[h][0:65, 0:w],
                        lhsT=vaug_sb[:, r * 65:(r + 1) * 65],
                        rhs=p_sb[:, h * w:(h + 1) * w],
                        start=(kt == 0),
                        stop=(kt == KT_S - 1),
                    )

                pvq = []
                for kt in range(KT_S):
                    k0 = b * S + kt * 128
                    p_sb = pP.tile([128, 1024], f16, tag="p",
                                   name=f"p_{b}_{qc}_{kt}{htag}")
                    for h in range(HPC):
                        sc = psSc.tile([128, 512], f32, tag="sc",
                                       name=f"sc_{b}_{qc}_{kt}_{h}{htag}")
                        nc.tensor.matmul(
                            sc[0:128, 0:w],
                            lhsT=kT_sb[h * 64:(h + 1) * 64, k0:k0 + 128],
                            rhs=qT_sb[h * 64:(h + 1) * 64, q0:q0 + w],
                            start=True, stop=True,
                            tile_position=(h * 64, 0),
                        )
                        if h == 1 and kt in _DVE_H1_KT:
                            nc.vector.tensor_scalar(
                                p_sb.bitcast(i16)[:, h * w:(h + 1) * w],
                                sc[0:128, 0:w], _SCH_A, _SCH_B,
                                ALU.mult, ALU.add,
                            )
                        else:
                            nc.scalar.activation(
                                p_sb[:, h * w:(h + 1) * w], sc[0:128, 0:w],
                                AF.Exp, scale=0.125)
                        pvq.append((kt, h, p_sb))
                        if len(pvq) > 2:
                            emit_pv(*pvq.pop(0))
                for it in pvq:
                    emit_pv(*it)
                # normalize: denom row 64 -> recip -> broadcast -> multiply
                rps = psO.tile([128, 512], f32, tag="a",
                               name=f"rps_{b}_{qc}{htag}")
                for h in range(HPC):
                    nc.scalar.copy(rec_st[h * 64:h * 64 + 1, 0:w],
                                   ctx_ps[h][64:65, 0:w])
                nc.vector.reciprocal_approx_fast(rec_full[0:128, 0:w],
                                                 rec_st[0:128, 0:w])
                for h in range(HPC):
                    nc.vector.tensor_copy(rec16c[h * 64:h * 64 + 1, 0:w],
                                          rec_full[h * 64:h * 64 + 1, 0:w])
                nc.tensor.matmul(rps[0:128, 0:w], lhsT=ecomb[:],
                                 rhs=rec16c[0:128, 0:w],
                                 start=True, stop=True)
                rps_sb = pDen.tile([128, 512], f16, tag="d",
                                   name=f"rd_{b}_{qc}{htag}")
                nc.scalar.copy(rps_sb[0:128, 0:w], rps[0:128, 0:w])
                for h in range(HPC):
                    nc.vector.tensor_tensor(
                        ctxT_sb[h * 64:(h + 1) * 64, q0:q0 + w],
                        ctx_ps[h][0:64, 0:w],
                        rps_sb[h * 64:(h + 1) * 64, 0:w], ALU.mult,
                    )

            # ---- per-qc AllGather + column-sharded output projection ----
            # The last qc is gathered/projected in two 256-token halves to
            # shorten the end-of-kernel serial tail.
            gath = {}

            def emit_gather(b, qc, half=None):
                q0 = b * S + qc * 512
                w = 512 if half is None else 256
                if half:
                    q0 += 256
                tag = f"{b}_{qc}" + ("" if half is None else f"_{half}")
                g_in = dram.tile([128, w], f16, name=f"gin_{tag}")
                nc.sync.dma_start(g_in[:], ctxT_sb[:, q0:q0 + w])
                g_out = dram.tile([NCORES * 128, w], f16,
                                  addr_space="Shared", name=f"gout_{tag}")
                nc.gpsimd.collective_compute(
                    "AllGather", mybir.AluOpType.bypass,
                    replica_groups=[list(range(NCORES))],
                    ins=[g_in.opt()], outs=[g_out.opt()],
                )
                gath[(b, qc, half)] = g_out

            def emit_outproj(key):
                b, qc, half = key
                q0 = b * S + qc * 512
                w = 512 if half is None else 256
                if half:
                    q0 += 256
                g_out = gath[key]
                tag = f"{b}_{qc}" + ("" if half is None else f"_{half}")
                cg = pCg.tile([128, NKT, 512], f16, tag="cg",
                              name=f"cg_{tag}")
                nc.sync.dma_start(
                    cg[:, :, 0:w],
                    g_out.rearrange("(k p) q -> p k q", p=128),
                )
                po = psO.tile([128, 512], f32, tag="a", name=f"po_{tag}")
                for k in range(NKT):
                    nc.tensor.matmul(
                        po[:, 0:w],
                        lhsT=wo_sb[:, k * 128:(k + 1) * 128],
                        rhs=cg[:, k, 0:w],
                        start=(k == 0),
                        stop=(k == NKT - 1),
                    )
                out_sb = pOut.tile([128, 512], f32, tag="os",
                                   name=f"os_{tag}")
                nc.vector.tensor_scalar(out_sb[:, 0:w], po[:, 0:w], bo_sb,
                                        None, ALU.add)
                nc.sync.dma_start(yT[:, q0:q0 + w], out_sb[:, 0:w])

            # ---- schedule ----
            # QKV b0 fully, then b0 attention with b1's QKV interleaved.
            for proj in range(3):
                for ncx in range(QC_S):
                    emit_qkv_chunk(0, proj, ncx)
            emit_vaug(0)

            fillers = ([(1, 2, n) for n in range(QC_S)]
                       + [(1, 1, n) for n in range(QC_S)]
                       + [(1, 0, n) for n in range(QC_S)])
            per_qc = [3, 3, 3, 3]
            pending_outproj = []
            for qc in range(QC_S):
                emit_attention_qc(0, qc)
                emit_gather(0, qc)
                pending_outproj.append((0, qc, None))
                for _ in range(per_qc[qc]):
                    if fillers:
                        b_, p_, n_ = fillers.pop(0)
                        emit_qkv_chunk(b_, p_, n_)
                if qc == 2:
                    emit_vaug(1)
                if len(pending_outproj) > 1:
                    emit_outproj(pending_outproj.pop(0))
            for qc in range(QC_S):
                emit_attention_qc(1, qc)
                if qc < QC_S - 1:
                    emit_gather(1, qc)
                    pending_outproj.append((1, qc, None))
                else:
                    emit_gather(1, qc, 0)
                    emit_gather(1, qc, 1)
                    pending_outproj += [(1, qc, 0), (1, qc, 1)]
                if len(pending_outproj) > 1:
                    emit_outproj(pending_outproj.pop(0))
            for key in pending_outproj:
                emit_outproj(key)

    nc.finalize()
    return nc


def kernel(x, Wq, Wk, Wv, bq, bk, bv, Wo, bo):
    from concourse.bass_utils import run_bass_kernel_spmd

    if "nc" not in _CACHE:
        _CACHE["nc"] = _build_kernel()
    nc = _CACHE["nc"]

    # host-side prep
    xTh = np.ascontiguousarray(
        x.astype(np.float32).transpose(2, 0, 1).reshape(D, SQ)
    ).astype(np.float16)

    def pack_w(Wslice):
        # [D, M] -> [128, NKT*M] kt-major: out[p, kt*M+m] = Wslice[kt*128+p, m]
        M = Wslice.shape[1]
        return np.ascontiguousarray(
            Wslice.reshape(NKT, 128, M).transpose(1, 0, 2).reshape(128, NKT * M)
        ).astype(np.float16)

    in_maps = []
    for c in range(NCORES):
        hA, hB = HPC * c, HPC * c + 1
        wq_c = pack_w(np.concatenate([Wq[hA], Wq[hB]], axis=1))
        wk_c = pack_w(np.concatenate([Wk[hA], Wk[hB]], axis=1))
        wv_c = pack_w(np.concatenate([Wv[hA], Wv[hB]], axis=1))
        wo_c = pack_w(Wo[:, 128 * c:128 * (c + 1)])
        wpack_c = np.ascontiguousarray(
            np.concatenate([wq_c, wk_c, wv_c, wo_c], axis=1))
        bq_c = np.concatenate([bq[hA], bq[hB]]).reshape(128, 1)
        bk_c = np.concatenate([bk[hA], bk[hB]]).reshape(128, 1)
        bv_c = np.concatenate([bv[hA], bv[hB]]).reshape(128, 1)
        bo_c = bo[128 * c:128 * (c + 1)].reshape(128, 1)
        bpack_c = np.ascontiguousarray(
            np.concatenate([bq_c, bk_c, bv_c, bo_c],
                           axis=1)).astype(np.float32)
        in_maps.append({"xT": xTh, "wpack": wpack_c, "bpack": bpack_c})

    res = run_bass_kernel_spmd(nc, in_maps, core_ids=list(range(NCORES)))
    _CACHE["last_result"] = res
    # assemble: core c's yT [128, SQ] are output columns 128c..128c+127
    out = np.empty((B, S, D), dtype=np.float32)
    for c in range(NCORES):
        yt = res.results[c]["yT"]  # [128, SQ]
        out[:, :, 128 * c:128 * (c + 1)] = (
            yt.reshape(128, B, S).transpose(1, 2, 0)
        )
    return out
